# revision 1
# baseline (speedup 1.0000x reference)
"""Trainium2 Bass kernel for the LSTM autoencoder (nn_AELSTM).

Per core (batch-sharded 64 rows/core across 8 cores):
  - H-major ("transposed") LSTM state: hT/cT live as [128 part = H-chunk, (k,b)]
    so the recurrence needs no per-step transposes.
  - Weights are the stationary matmul operand; the per-step moving operand is
    hT [128, 64]. Gates accumulate gate-major in PSUM, grouped (f,i | g | o)
    so each step needs only 3 gate activations + tanh(c).
  - Encoder input projections x@W_ih.T are batched 4 timesteps at a time into
    the PSUM gate tiles ahead of the recurrence; biases ride in via small
    selector matmuls (K=4/2).
  - Decoder input is constant over time: its projection (+bias) is computed
    once into SBUF bf16 and re-played into each step's PSUM bank with an
    identity matmul (N=512), prefilled 2 steps ahead of the recurrence.
  - Output projection is batched 4 steps at a time from the bf16 h history
    (history writes on GPSIMD); out_b rides in via a K=1 matmul and the
    result DMAs to DRAM straight from PSUM.
  - All PE-facing data is bf16 (fp32 PSUM accumulation); cell state c stays
    fp32 in SBUF. Numpy-simulated end-to-end rel-l2 error ~2.5e-3.
  - Host does every layout transpose; the device does none.
"""
import sys
import os

for _p in ('/opt/trn_rl_repo', '/root/.axon_site/_ro/trn_rl_repo'):
    if os.path.isdir(_p) and _p not in sys.path:
        sys.path.insert(0, _p)

import numpy as np
import ml_dtypes

import concourse.bass as bass
from concourse import bacc
import concourse.mybir as mybir
import concourse.tile as tile
from concourse.bass_utils import run_bass_kernel_spmd
from concourse.tile import add_dep_helper

bf16 = ml_dtypes.bfloat16
FP32 = mybir.dt.float32
BF16 = mybir.dt.bfloat16

B, T, D, H = 512, 512, 128, 256
GORDER = os.environ.get("K_GORDER", "0") == "1"
AORDER = os.environ.get("K_AORDER", "0") == "1"
SPREAD = os.environ.get("K_SPREAD", "0") == "1"
CBF16 = os.environ.get("K_CBF16", "0") == "1"
FWAIT = os.environ.get("K_FWAIT", "0") == "1"
FGATE = os.environ.get("K_FGATE", "1") == "1"
ESTART = float(os.environ.get("K_ESTART", "22000"))   # ns, enc start estimate
ESTEP = float(os.environ.get("K_ESTEP", "2000"))      # ns, enc step estimate
CDT_ENV = None
NC = 8           # cores
BL = B // NC     # 64 batch rows per core
S = 4            # timesteps per pipeline phase
XCH = 8          # x DMA chunks
TCH = T // XCH   # timesteps per x chunk
PERM = [2, 3, 0, 1, 4, 5, 6, 7]   # enc psum chunk j -> original gate chunk
                                  # (orig order i,f,g,o -> enc psum order f,i,g,o)
PERM_D = [2, 3, 0, 1, 6, 7, 4, 5]  # dec psum order f,i,o,g

Sig = mybir.ActivationFunctionType.Sigmoid
Tanh = mybir.ActivationFunctionType.Tanh


def _step_tail(nc, work, fiact, gact, oact, c_prev):
    """c = f*c + i*g ; h = o*tanh(c). fiact = [f | i] [128, 256]."""
    t1 = work.tile([128, 128], BF16, tag="t1")
    nc.vector.tensor_tensor(t1[:], fiact[:, 128:256], gact,
                            mybir.AluOpType.mult)
    CDT = BF16 if CBF16 else FP32
    t2 = work.tile([128, 128], CDT, tag="t2")
    nc.vector.tensor_tensor(t2[:], fiact[:, 0:128], c_prev[:],
                            mybir.AluOpType.mult)
    c_new = work.tile([128, 128], CDT, tag="c")
    nc.vector.tensor_tensor(c_new[:], t2[:], t1[:], mybir.AluOpType.add)
    tc_ = work.tile([128, 128], BF16, tag="tanhc")
    nc.scalar.activation(tc_[:], c_new[:], Tanh)
    h_new = work.tile([128, 128], BF16, tag="h")
    nc.vector.tensor_tensor(h_new[:], oact, tc_[:], mybir.AluOpType.mult)
    return h_new, c_new


def build():
    nc = bacc.Bacc(None, target_bir_lowering=False)
    P = nc.declare_dram_parameter

    x_d = P("x", [128, T, BL], BF16, isOutput=False)           # [d, t, b]
    eWih = P("eWih", [128, 1024], BF16, isOutput=False)        # enc W_ih.T (perm)
    eWhh0 = P("eWhh0", [128, 1024], BF16, isOutput=False)      # enc W_hh.T k0 (perm)
    eWhh1 = P("eWhh1", [128, 1024], BF16, isOutput=False)
    bias4fi = P("bias4fi", [4, 256], BF16, isOutput=False)     # [j4, {enc|dec}*128+p]
    bias2g = P("bias2g", [2, 256], BF16, isOutput=False)
    bias2o = P("bias2o", [2, 256], BF16, isOutput=False)
    sel4 = P("sel4", [4, S * 256], BF16, isOutput=False)
    sel2 = P("sel2", [2, S * 128], BF16, isOutput=False)
    dbias8 = P("dbias8", [8, 128], BF16, isOutput=False)       # dec bias (PERM_D)
    sel8 = P("sel8", [8, 512], BF16, isOutput=False)
    dWih0 = P("dWih0", [128, 1024], BF16, isOutput=False)      # (perm)
    dWih1 = P("dWih1", [128, 1024], BF16, isOutput=False)
    dWhh0 = P("dWhh0", [128, 1024], BF16, isOutput=False)      # (perm)
    dWhh1 = P("dWhh1", [128, 1024], BF16, isOutput=False)
    ident = P("ident", [128, 128], BF16, isOutput=False)
    efcT0 = P("efcT0", [128, 128], BF16, isOutput=False)       # enc_fc_W.T k0
    efcT1 = P("efcT1", [128, 128], BF16, isOutput=False)
    efcb = P("efcb", [128, 1], FP32, isOutput=False)
    dfcT = P("dfcT", [128, 256], BF16, isOutput=False)         # dec_fc_W.T
    dfcb = P("dfcb", [128, 2], FP32, isOutput=False)
    oWT0 = P("oWT0", [128, 128], BF16, isOutput=False)         # out_W.T k0
    oWT1 = P("oWT1", [128, 128], BF16, isOutput=False)
    obrow = P("obrow", [1, 128], BF16, isOutput=False)         # out_b as row
    ones = P("ones", [1, S * 64], BF16, isOutput=False)
    out_d = P("out", [128, T, BL], FP32, isOutput=True)        # [d, t, b]
    DEBUG = os.environ.get("K_DEBUG", "0") == "1"
    if DEBUG:
        dbg_hlast = P("dbg_hlast", [128, 128], FP32, isOutput=True)
        dbg_z = P("dbg_z", [128, 64], FP32, isOutput=True)
        dbg_dv = P("dbg_dv", [128, 128], FP32, isOutput=True)
        dbg_xgd = P("dbg_xgd", [128, 512], FP32, isOutput=True)
        dbg_hd = P("dbg_hd", [128, 128, 8], FP32, isOutput=True)  # dec h t=0..7
        dbg_xgdb = P("dbg_xgdb", [128, 512], FP32, isOutput=True)
        dbg_g0 = P("dbg_g0", [128, 512], FP32, isOutput=True)      # enc gates t=0 pre-act
        dbg_he = P("dbg_he", [128, 128, 8], FP32, isOutput=True)   # enc h t=0..7

    with tile.TileContext(nc) as tc:
        with tc.tile_pool(name="xpool", bufs=1) as xpool, \
             tc.tile_pool(name="wpool", bufs=1) as wpool, \
             tc.tile_pool(name="work", bufs=3) as work, \
             tc.tile_pool(name="acts", bufs=3) as act_pool, \
             tc.tile_pool(name="hist", bufs=2) as hist_pool:

            def wtile(param, shape, dt=BF16):
                t_ = wpool.tile(shape, dt, tag=param.name)
                nc.sync.dma_start(out=t_[:], in_=param[:])
                return t_

            # ---- weights / constants
            eWih_s = wtile(eWih, [128, 1024])
            eWhh_s = [wtile(eWhh0, [128, 1024]), wtile(eWhh1, [128, 1024])]
            bias4fi_s = wtile(bias4fi, [4, 256])
            bias2g_s = wtile(bias2g, [2, 256])
            bias2o_s = wtile(bias2o, [2, 256])
            sel4_s = wtile(sel4, [4, S * 256])
            sel2_s = wtile(sel2, [2, S * 128])
            dWih_s = [wtile(dWih0, [128, 1024]), wtile(dWih1, [128, 1024])]
            dbias8_s = wtile(dbias8, [8, 128])
            sel8_s = wtile(sel8, [8, 512])
            dWhh_s = [wtile(dWhh0, [128, 1024]), wtile(dWhh1, [128, 1024])]
            ident_s = wtile(ident, [128, 128])
            efcT_s = [wtile(efcT0, [128, 128]), wtile(efcT1, [128, 128])]
            efcb_s = wtile(efcb, [128, 1], FP32)
            dfcT_s = wtile(dfcT, [128, 256])
            dfcb_s = wtile(dfcb, [128, 2], FP32)
            oWT_s = [wtile(oWT0, [128, 128]), wtile(oWT1, [128, 128])]
            obrow_s = wtile(obrow, [1, 128])
            ones_s = wtile(ones, [1, S * 64])

            # ---- x prefetch; first chunk small so the encoder starts early
            xt = []          # list of (t_start, t_len, tile)
            bounds = [0, TCH] + [TCH * i for i in range(2, XCH + 1)]
            for ci in range(len(bounds) - 1):
                t0c, t1c = bounds[ci], bounds[ci + 1]
                t_ = xpool.tile([128, t1c - t0c, BL], BF16, tag=f"x{ci}",
                                name=f"xch{ci}")
                nc.sync.dma_start(out=t_[:], in_=x_d[:, t0c:t1c, :])
                xt.append((t0c, t1c, t_))

            def x_slice(t0, n):
                for (a, b, tl) in xt:
                    if a <= t0 and t0 + n <= b:
                        return tl[:, t0 - a:t0 - a + n, :]
                raise AssertionError(f"x slice {t0}+{n} crosses chunks")

            REPEAT = int(os.environ.get("K_REPEAT", "1"))
            for _rep in range(REPEAT):
                emit_model(nc, tc, locals())

    nc.finalize()
    return nc


def emit_model(nc, tc, env):
    for k, v in env.items():
        globals()["_E_" + k] = v
    class _G:
        def __getattr__(self, k):
            return globals()["_E_" + k]
    g = _G()
    (work, act_pool, hist_pool, xt, eWih_s, eWhh_s, bias4fi_s, bias2g_s,
     bias2o_s, sel4_s, sel2_s, dWih_s, dWhh_s, dbias8_s, sel8_s, ident_s,
     efcT_s, efcb_s, dfcT_s, dfcb_s, oWT_s, obrow_s, ones_s, out_d) = (
        g.work, g.act_pool, g.hist_pool, g.xt, g.eWih_s, g.eWhh_s,
        g.bias4fi_s, g.bias2g_s, g.bias2o_s, g.sel4_s, g.sel2_s, g.dWih_s,
        g.dWhh_s, g.dbias8_s, g.sel8_s, g.ident_s, g.efcT_s, g.efcb_s,
        g.dfcT_s, g.dfcb_s, g.oWT_s, g.obrow_s, g.ones_s, g.out_d)
    x_slice = g.x_slice
    try:
        DEBUG = g.DEBUG
    except KeyError:
        DEBUG = False
    if DEBUG:
        (dbg_hlast, dbg_z, dbg_dv, dbg_xgd, dbg_hd, dbg_xgdb, dbg_g0,
         dbg_he) = (g.dbg_hlast, g.dbg_z, g.dbg_dv, g.dbg_xgd, g.dbg_hd,
                    g.dbg_xgdb, g.dbg_g0, g.dbg_he)

    h0 = work.tile([128, 128], BF16, tag="h")
    c0 = work.tile([128, 128], BF16 if CBF16 else FP32, tag="c")
    nc.vector.memset(h0[:], 0.0)
    nc.vector.memset(c0[:], 0.0)
    if True:
        if True:
            # ================= encoder =================
            # psum per phase: fi tile [128, S*256] (2 banks), g [128, S*128],
            # o [128, S*128] -> 4 banks x bufs=2 = 8 banks
            with tc.tile_pool(name="pfi", bufs=2, space="PSUM") as pfi, \
                 tc.tile_pool(name="pgg", bufs=2, space="PSUM") as pgg, \
                 tc.tile_pool(name="poo", bufs=2, space="PSUM") as poo:

                def enc_fill_ops(ph):
                    """Allocate phase tiles; return (tiles, list of emit-thunks)."""
                    t0 = ph * S
                    xr = x_slice(t0, S)
                    tfi = pfi.tile([128, S * 256], FP32, tag="fi", name=f"fi{ph}")
                    tg = pgg.tile([128, S * 128], FP32, tag="gg", name=f"gg{ph}")
                    to = poo.tile([128, S * 128], FP32, tag="oo", name=f"oo{ph}")
                    fir = tfi[:].rearrange("p (t j b) -> p t j b", t=S, j=4, b=BL)
                    gr = tg[:].rearrange("p (t j b) -> p t j b", t=S, j=2, b=BL)
                    orr = to[:].rearrange("p (t j b) -> p t j b", t=S, j=2, b=BL)
                    ops = []
                    # PSUM rule: exactly ONE start=True matmul per 2KB bank,
                    # covering the whole bank (start resets the whole
                    # zero-region's accumulation state). tg/to = 1 bank each;
                    # tfi = 2 banks -> 2 bias matmuls of 512 cols.
                    ops.append(lambda: nc.tensor.matmul(
                        tg[:], lhsT=bias2g_s[:, 0:128], rhs=sel2_s[:],
                        start=True, stop=False, skip_group_check=True))
                    for j in range(2):
                        ops.append(lambda j=j: nc.tensor.matmul(
                            gr[:, :, j, :],
                            lhsT=eWih_s[:, (4 + j) * 128:(4 + j) * 128 + 128],
                            rhs=xr, start=False, stop=False,
                            skip_group_check=True))
                    for i in range(2):
                        ops.append(lambda i=i: nc.tensor.matmul(
                            tfi[:, i * 512:i * 512 + 512],
                            lhsT=bias4fi_s[:, 0:128],
                            rhs=sel4_s[:, i * 512:i * 512 + 512],
                            start=True, stop=False, skip_group_check=True))
                    for j in range(4):
                        ops.append(lambda j=j: nc.tensor.matmul(
                            fir[:, :, j, :],
                            lhsT=eWih_s[:, j * 128:j * 128 + 128],
                            rhs=xr, start=False, stop=False,
                            skip_group_check=True))
                    ops.append(lambda: nc.tensor.matmul(
                        to[:], lhsT=bias2o_s[:, 0:128], rhs=sel2_s[:],
                        start=True, stop=False, skip_group_check=True))
                    for j in range(2):
                        ops.append(lambda j=j: nc.tensor.matmul(
                            orr[:, :, j, :],
                            lhsT=eWih_s[:, (6 + j) * 128:(6 + j) * 128 + 128],
                            rhs=xr, start=False, stop=False,
                            skip_group_check=True))
                    return (tfi, tg, to), ops

                PREF = 1
                phases = T // S
                gate_tiles = [None] * phases
                pending_fill = []
                for ph in range(PREF):
                    gate_tiles[ph], ops = enc_fill_ops(ph)
                    for op in ops:
                        op()
                h_prev, c_prev = h0, c0
                for t in range(T):
                    ph, tp = divmod(t, S)
                    if tp == 0 and ph + PREF < phases:
                        gate_tiles[ph + PREF], pending_fill = \
                            enc_fill_ops(ph + PREF)
                    tfi, tg, to = gate_tiles[ph]
                    def _mm_g():
                        for j in range(2):
                            for k in range(2):
                                nc.tensor.matmul(
                                    tg[:, tp * 128 + j * 64:tp * 128 + j * 64 + 64],
                                    lhsT=eWhh_s[k][:, (4 + j) * 128:(4 + j) * 128 + 128],
                                    rhs=h_prev[:, k * 64:k * 64 + 64],
                                    start=False, stop=(k == 1),
                                    skip_group_check=True)
                    def _mm_fi():
                        for j in range(4):
                            for k in range(2):
                                nc.tensor.matmul(
                                    tfi[:, tp * 256 + j * 64:tp * 256 + j * 64 + 64],
                                    lhsT=eWhh_s[k][:, j * 128:j * 128 + 128],
                                    rhs=h_prev[:, k * 64:k * 64 + 64],
                                    start=False, stop=(k == 1),
                                    skip_group_check=True)
                    def _mm_o():
                        for j in range(2):
                            for k in range(2):
                                nc.tensor.matmul(
                                    to[:, tp * 128 + j * 64:tp * 128 + j * 64 + 64],
                                    lhsT=eWhh_s[k][:, (6 + j) * 128:(6 + j) * 128 + 128],
                                    rhs=h_prev[:, k * 64:k * 64 + 64],
                                    start=False, stop=(k == 1),
                                    skip_group_check=True)
                    if GORDER:
                        _mm_g(); _mm_fi(); _mm_o()
                    else:
                        _mm_fi(); _mm_g(); _mm_o()
                    # spread next-phase fill MMs across this phase's steps
                    nops = len(pending_fill)
                    if nops:
                        if SPREAD:
                            # spread over tp=0..S-2; keep the boundary step
                            # (tp=S-1) free of fills so the next step's
                            # chain-critical matmuls aren't queued behind them
                            W = S - 1
                            lo = (nops * min(tp, W)) // W
                            hi = (nops * min(tp + 1, W)) // W
                            wait_ms = (ESTART + (ph * S + tp) * ESTEP) / 1e6
                            with tc.tile_wait_until(wait_ms, enable=FWAIT):
                                for op in pending_fill[lo:hi]:
                                    op()
                            if tp == S - 1:
                                pending_fill = []
                        else:
                            for op in pending_fill:
                                op()
                            pending_fill = []
                    if DEBUG and t == 0:
                        dbgg = work.tile([128, 512], FP32, tag="dbg", name="dbgg")
                        nc.vector.tensor_copy(dbgg[:, 0:256], tfi[:, 0:256])
                        nc.vector.tensor_copy(dbgg[:, 256:384], tg[:, 0:128])
                        nc.vector.tensor_copy(dbgg[:, 384:512], to[:, 0:128])
                        nc.sync.dma_start(out=dbg_g0[:], in_=dbgg[:])
                    gact = act_pool.tile([128, 128], BF16, tag="gact")
                    fiact = act_pool.tile([128, 256], BF16, tag="fiact")
                    oact = act_pool.tile([128, 128], BF16, tag="oact")
                    nops = len(pending_fill)
                    if nops:
                        if SPREAD:
                            W = S - 1
                            lo = (nops * min(tp, W)) // W
                            hi = (nops * min(tp + 1, W)) // W
                            for op in pending_fill[lo:hi]:
                                op()
                            if tp == S - 1:
                                pending_fill = []
                        else:
                            for op in pending_fill:
                                op()
                            pending_fill = []
                    if AORDER:
                        nc.scalar.activation(gact[:],
                                             tg[:, tp * 128:tp * 128 + 128], Tanh)
                        nc.scalar.activation(fiact[:],
                                             tfi[:, tp * 256:tp * 256 + 256], Sig)
                    else:
                        nc.scalar.activation(fiact[:],
                                             tfi[:, tp * 256:tp * 256 + 256], Sig)
                        nc.scalar.activation(gact[:],
                                             tg[:, tp * 128:tp * 128 + 128], Tanh)
                    sigo_ins = nc.scalar.activation(
                        oact[:], to[:, tp * 128:tp * 128 + 128], Sig)
                    h_prev, c_prev = _step_tail(nc, work, fiact,
                                                gact[:], oact[:], c_prev)
                    if DEBUG and t < 8:
                        dbghe = work.tile([128, 128], FP32, tag="dbg",
                                          name=f"dbghe{t}")
                        nc.vector.tensor_copy(dbghe[:], h_prev[:])
                        nc.sync.dma_start(out=dbg_he[:, :, t], in_=dbghe[:])
                    if tp == S - 1:
                        gate_tiles[ph] = None
                h_last = h_prev

            # ================= bottleneck =================
            with tc.tile_pool(name="psfc", bufs=2, space="PSUM") as psfc:
                z_ps = psfc.tile([128, 64], FP32, tag="fc")
                for k in range(2):
                    nc.tensor.matmul(z_ps[:], lhsT=efcT_s[k][:],
                                     rhs=h_last[:, k * 64:k * 64 + 64],
                                     start=(k == 0), stop=(k == 1),
                                     skip_group_check=True)
                z_sb = work.tile([128, 64], BF16, tag="z")
                nc.vector.tensor_scalar_add(z_sb[:], z_ps[:], efcb_s[:])
                dv_sb = work.tile([128, 128], BF16, tag="dv")
                for m in range(2):
                    dv_ps = psfc.tile([128, 64], FP32, tag="fc", name=f"dvps{m}")
                    nc.tensor.matmul(dv_ps[:],
                                     lhsT=dfcT_s[:, m * 128:m * 128 + 128],
                                     rhs=z_sb[:], start=True, stop=True,
                                     skip_group_check=True)
                    nc.vector.tensor_scalar_add(dv_sb[:, m * 64:m * 64 + 64],
                                                dv_ps[:], dfcb_s[:, m:m + 1])
                # xg_dec (+bias) -> SBUF bf16 [128, (j,b)], permuted gate order
                xgd_ps = psfc.tile([128, 512], FP32, tag="xgd")
                nc.tensor.matmul(xgd_ps[:], lhsT=dbias8_s[:], rhs=sel8_s[:],
                                 start=True, stop=False, skip_group_check=True)
                if DEBUG:
                    dbgb = work.tile([128, 512], FP32, tag="dbg", name="dbgb")
                    nc.vector.tensor_copy(dbgb[:], xgd_ps[:])
                    nc.sync.dma_start(out=dbg_xgdb[:], in_=dbgb[:])
                for j in range(8):
                    for k in range(2):
                        nc.tensor.matmul(
                            xgd_ps[:, j * 64:j * 64 + 64],
                            lhsT=dWih_s[k][:, j * 128:j * 128 + 128],
                            rhs=dv_sb[:, k * 64:k * 64 + 64],
                            start=False, stop=(k == 1), skip_group_check=True)
                xgd_sb = work.tile([128, 512], BF16, tag="xgd")
                nc.vector.tensor_copy(xgd_sb[:], xgd_ps[:])
                # duplicated per-2-steps variants for full-bank identity fills
                xgd2_fi = work.tile([128, 512], BF16, tag="xgd2fi")
                xgd2_og = work.tile([128, 512], BF16, tag="xgd2og")
                for r in range(2):
                    nc.vector.tensor_copy(xgd2_fi[:, r * 256:r * 256 + 256],
                                          xgd_sb[:, 0:256])
                    nc.vector.tensor_copy(xgd2_og[:, r * 256:r * 256 + 256],
                                          xgd_sb[:, 256:512])
                if DEBUG:
                    dbgt = work.tile([128, 512], FP32, tag="dbg", name="dbg1")
                    nc.vector.tensor_copy(dbgt[:, 0:128], h_last[:])
                    nc.sync.dma_start(out=dbg_hlast[:], in_=dbgt[:, 0:128])
                    nc.vector.tensor_copy(dbgt[:, 128:192], z_sb[:])
                    nc.sync.dma_start(out=dbg_z[:], in_=dbgt[:, 128:192])
                    nc.vector.tensor_copy(dbgt[:, 192:320], dv_sb[:])
                    nc.sync.dma_start(out=dbg_dv[:], in_=dbgt[:, 192:320])
                    dbgt2 = work.tile([128, 512], FP32, tag="dbg", name="dbg2")
                    nc.vector.tensor_copy(dbgt2[:], xgd_sb[:])
                    nc.sync.dma_start(out=dbg_xgd[:], in_=dbgt2[:])

            # ================= decoder =================
            # per-phase (S2=2 steps) split PSUM tiles so sigma(fi) only waits
            # on the fi matmuls: tfi [128, 2*256] (f,i), tgo [128, 2*256]
            # (o at +0:128, g at +128:256 per step). xgd_sb cols (f,i,o,g).
            S2 = 2
            with tc.tile_pool(name="ptfi", bufs=3, space="PSUM") as ptfi, \
                 tc.tile_pool(name="ptgo", bufs=3, space="PSUM") as ptgo, \
                 tc.tile_pool(name="prec", bufs=2, space="PSUM") as prec:
                DPREF = 2
                phases2 = T // S2
                dtiles = [None] * phases2
                pending_dfill = []

                def dec_fill_ops(ph2):
                    tfi = ptfi.tile([128, S2 * 256], FP32, tag="tfi",
                                    name=f"tfi{ph2}")
                    tgo = ptgo.tile([128, S2 * 256], FP32, tag="tgo",
                                    name=f"tgo{ph2}")
                    ops = [
                        lambda: nc.tensor.matmul(
                            tfi[:], lhsT=ident_s[:], rhs=xgd2_fi[:],
                            start=True, stop=False, skip_group_check=True),
                        lambda: nc.tensor.matmul(
                            tgo[:], lhsT=ident_s[:], rhs=xgd2_og[:],
                            start=True, stop=False, skip_group_check=True),
                    ]
                    return (tfi, tgo), ops

                for ph2 in range(DPREF):
                    dtiles[ph2], ops = dec_fill_ops(ph2)
                    for op in ops:
                        op()
                pending_outproj = None
                h_prev = work.tile([128, 128], BF16, tag="h")
                c_prev = work.tile([128, 128], BF16 if CBF16 else FP32, tag="c")
                nc.vector.memset(h_prev[:], 0.0)
                nc.vector.memset(c_prev[:], 0.0)
                hist = {}
                for t in range(T):
                    ph2, tp = divmod(t, S2)
                    oph, otp = divmod(t, S)
                    if tp == 0 and ph2 + DPREF < phases2:
                        dtiles[ph2 + DPREF], pending_dfill = \
                            dec_fill_ops(ph2 + DPREF)
                    tfi, tgo = dtiles[ph2]
                    # fi chunks (j 0..3), then o (4,5), then g (6,7; last so
                    # tanh(g) fires as soon as the tgo tile quiesces)
                    for j in range(4):
                        for k in range(2):
                            nc.tensor.matmul(
                                tfi[:, tp * 256 + j * 64:tp * 256 + j * 64 + 64],
                                lhsT=dWhh_s[k][:, j * 128:j * 128 + 128],
                                rhs=h_prev[:, k * 64:k * 64 + 64],
                                start=False, stop=(k == 1),
                                skip_group_check=True)
                    for j in range(4, 6):
                        for k in range(2):
                            nc.tensor.matmul(
                                tgo[:, tp * 256 + (j - 4) * 64:
                                     tp * 256 + (j - 4) * 64 + 64],
                                lhsT=dWhh_s[k][:, j * 128:j * 128 + 128],
                                rhs=h_prev[:, k * 64:k * 64 + 64],
                                start=False, stop=(k == 1),
                                skip_group_check=True)
                    for j in range(6, 8):
                        for k in range(2):
                            nc.tensor.matmul(
                                tgo[:, tp * 256 + 128 + (j - 6) * 64:
                                     tp * 256 + 128 + (j - 6) * 64 + 64],
                                lhsT=dWhh_s[k][:, j * 128:j * 128 + 128],
                                rhs=h_prev[:, k * 64:k * 64 + 64],
                                start=False, stop=(k == 1),
                                skip_group_check=True)
                    nops = len(pending_dfill)
                    if nops:
                        if tp == 0:
                            for op in pending_dfill:
                                op()
                        if tp == S2 - 1:
                            pending_dfill = []
                    if pending_outproj is not None:
                        pending_outproj()
                        pending_outproj = None
                    fiact = act_pool.tile([128, 256], BF16, tag="fiact")
                    gact = act_pool.tile([128, 128], BF16, tag="gact")
                    oact = act_pool.tile([128, 128], BF16, tag="oact")
                    nc.scalar.activation(fiact[:],
                                         tfi[:, tp * 256:tp * 256 + 256], Sig)
                    nc.scalar.activation(gact[:],
                                         tgo[:, tp * 256 + 128:tp * 256 + 256],
                                         Tanh)
                    nc.scalar.activation(oact[:],
                                         tgo[:, tp * 256:tp * 256 + 128], Sig)
                    h_prev, c_prev = _step_tail(nc, work, fiact,
                                                gact[:], oact[:], c_prev)
                    if DEBUG and t < 8:
                        dbgh = work.tile([128, 128], FP32, tag="dbg",
                                         name=f"dbgh{t}")
                        nc.vector.tensor_copy(dbgh[:], h_prev[:])
                        nc.sync.dma_start(out=dbg_hd[:, :, t], in_=dbgh[:])
                    if otp == 0:
                        hist[0] = hist_pool.tile([128, S * 64], BF16,
                                                 tag="hk0", name="histk0")
                        hist[1] = hist_pool.tile([128, S * 64], BF16,
                                                 tag="hk1", name="histk1")
                    for k in range(2):
                        nc.gpsimd.tensor_copy(hist[k][:, otp * 64:otp * 64 + 64],
                                              h_prev[:, k * 64:k * 64 + 64])
                    if otp == S - 1:
                        def mk_outproj(oph=oph, h0=hist[0], h1=hist[1]):
                            def emit():
                                rec_ps = prec.tile([128, S * 64], FP32,
                                                   tag="rec", name=f"rec{oph}")
                                nc.tensor.matmul(rec_ps[:], lhsT=obrow_s[:],
                                                 rhs=ones_s[:], start=True,
                                                 stop=False,
                                                 skip_group_check=True)
                                for k in range(2):
                                    nc.tensor.matmul(
                                        rec_ps[:], lhsT=oWT_s[k][:],
                                        rhs=(h0 if k == 0 else h1)[:],
                                        start=False, stop=(k == 1),
                                        skip_group_check=True)
                                ro = work.tile([128, S * 64], FP32, tag="ro",
                                               name=f"ro{oph}")
                                nc.vector.tensor_copy(ro[:], rec_ps[:])
                                nc.sync.dma_start(
                                    out=out_d[:, oph * S:(oph + 1) * S, :],
                                    in_=ro[:].rearrange("p (t b) -> p t b",
                                                        t=S, b=BL))
                            return emit
                        pending_outproj = mk_outproj()
                if pending_outproj is not None:
                    pending_outproj()


def _prep_core_inputs(inputs, core):
    f = np.float32
    x = np.asarray(inputs['x'], f)[core * BL:(core + 1) * BL]  # [64, 512, 128]
    xT = np.ascontiguousarray(x.transpose(2, 1, 0)).astype(bf16)  # [d, t, b]

    def bT(a):
        return np.ascontiguousarray(np.asarray(a, f).T).astype(bf16)

    def permT(wT, perm=PERM):  # permute gate chunks of [K, 1024] weight.T
        cols = np.concatenate([wT[:, p * 128:(p + 1) * 128] for p in perm],
                              axis=1)
        return np.ascontiguousarray(cols)

    eWihT = permT(bT(inputs['enc_W_ih']))   # [128, 1024]
    eWhhT = permT(bT(inputs['enc_W_hh']))   # [256, 1024]
    dWihT = permT(bT(inputs['dec_W_ih']), PERM_D)
    dWhhT = permT(bT(inputs['dec_W_hh']), PERM_D)
    ebias = (np.asarray(inputs['enc_b_ih'], f) +
             np.asarray(inputs['enc_b_hh'], f)).reshape(8, 128)[PERM]
    dbias = (np.asarray(inputs['dec_b_ih'], f) +
             np.asarray(inputs['dec_b_hh'], f)).reshape(8, 128)[PERM_D]
    bias4fi = np.concatenate([ebias[0:4], dbias[0:4]], axis=1).astype(bf16)
    bias2g = np.concatenate([ebias[4:6], dbias[6:8]], axis=1).astype(bf16)
    bias2o = np.concatenate([ebias[6:8], dbias[4:6]], axis=1).astype(bf16)
    sel4 = np.zeros((4, S * 256), f)
    for t in range(S):
        for j in range(4):
            sel4[j, t * 256 + j * 64:t * 256 + j * 64 + 64] = 1.0
    sel2 = np.zeros((2, S * 128), f)
    for t in range(S):
        for j in range(2):
            sel2[j, t * 128 + j * 64:t * 128 + j * 64 + 64] = 1.0
    dbias8 = dbias.astype(bf16)                 # [8, 128] in PERM_D order
    sel8 = np.zeros((8, 512), f)
    for j in range(8):
        sel8[j, j * 64:j * 64 + 64] = 1.0
    efcT = bT(inputs['enc_fc_W'])           # [256, 128]
    dfcT = bT(inputs['dec_fc_W'])           # [128, 256]
    oWT = bT(inputs['out_W'])               # [256, 128]
    return {
        "x": xT,
        "eWih": eWihT,
        "eWhh0": np.ascontiguousarray(eWhhT[0:128]),
        "eWhh1": np.ascontiguousarray(eWhhT[128:256]),
        "bias4fi": bias4fi, "bias2g": bias2g, "bias2o": bias2o,
        "sel4": sel4.astype(bf16), "sel2": sel2.astype(bf16),
        "dbias8": dbias8, "sel8": sel8.astype(bf16),
        "dWih0": np.ascontiguousarray(dWihT[0:128]),
        "dWih1": np.ascontiguousarray(dWihT[128:256]),
        "dWhh0": np.ascontiguousarray(dWhhT[0:128]),
        "dWhh1": np.ascontiguousarray(dWhhT[128:256]),
        "ident": np.eye(128, dtype=bf16),
        "efcT0": np.ascontiguousarray(efcT[0:128]),
        "efcT1": np.ascontiguousarray(efcT[128:256]),
        "efcb": np.asarray(inputs['enc_fc_b'], f).reshape(128, 1),
        "dfcT": dfcT,
        "dfcb": np.ascontiguousarray(
            np.asarray(inputs['dec_fc_b'], f).reshape(2, 128).T),
        "oWT0": np.ascontiguousarray(oWT[0:128]),
        "oWT1": np.ascontiguousarray(oWT[128:256]),
        "obrow": np.asarray(inputs['out_b'], f).reshape(1, 128).astype(bf16),
        "ones": np.ones((1, S * 64), bf16),
    }


_NC_CACHE = None


def kernel(**inputs):
    global _NC_CACHE
    if _NC_CACHE is None:
        _NC_CACHE = build()
    nc = _NC_CACHE
    in_maps = [_prep_core_inputs(inputs, c) for c in range(NC)]
    res = run_bass_kernel_spmd(nc, in_maps, core_ids=list(range(NC)))
    outs = []
    for c in range(NC):
        o = res.results[c]["out"]                      # [128 d, 512 t, 64 b]
        outs.append(np.ascontiguousarray(o.transpose(2, 1, 0)))
    return np.concatenate(outs, axis=0).astype(np.float32)


if __name__ == "__main__":
    import time
    t0 = time.time()
    nc = build()
    print("built in", time.time() - t0, "s")

